# revision 18
# baseline (speedup 1.0000x reference)
"""Masked dot-product attention (B=16, Lq=Lk=2048, D=Dv=256, fp32) on 8 trn2 cores.

Strategy (data-parallel over batch, 2 batches/core):
  - Host pre-transposes Q,K to [d, seq] layout (bf16) so both matmuls run
    without any on-device transposes:
      S^T[k,q]  = (K^T chunk).T @ (Q^T chunk)   (contract d, 2 chunks of 128)
      P^T[k,q]  = exp(S^T/16 + maskbias[k])     (ACT, per-partition bias folds
                                                 the valid_len mask: -1e30 -> 0)
      O[q,v]    = sum_kb (P^T slice).T @ V'[kb] (accumulate in PSUM)
    where V' = [V | ones] so column 256 of O accumulates sum_k exp = softmax
    denominator; final normalize is a per-partition reciprocal-multiply.
  - Scores are O(1) (inputs ~N(0,1), /sqrt(256)), so exp without max-subtraction
    is numerically safe in fp32; matmul inputs in bf16 (fp32 PSUM accumulate).
"""

import contextlib
import os

import numpy as np
import ml_dtypes

import concourse.bass as bass
import concourse.bacc as bacc
import concourse.tile as tile
from concourse import mybir
from concourse.bass_utils import run_bass_kernel_spmd

B, LQ, LK, D, DV = 16, 2048, 2048, 256, 256
NCORES = 8
BPC = B // NCORES          # batches per core
NKB = LK // 128            # 16 k-blocks of 128
QT = 512                   # q tile (one PSUM bank of fp32)
NQT = LQ // QT             # 4
NQS = QT // 128            # 4 q sub-tiles per q tile
NDC = D // 128             # 2 contraction chunks

BF16 = mybir.dt.bfloat16
F32 = mybir.dt.float32
EXP = mybir.ActivationFunctionType.Exp

_progs = {}

# Sparse mode: k-block groups past the first are wrapped in runtime
# `If(nkb[b] > g*GS)` so fully-masked tails are skipped on-device.
GS = 4  # k-blocks per gated group


def _build_program(
    timing_loop: bool = False,
    sparse: bool = False,
    ps_bufs: int = 3,
    prefetch: bool = False,
):
    """Build the SPMD program. With timing_loop=True an extra int32 input
    `riter` [1,1] repeats the whole body riter times on-device (wall-clock
    slope timing — no NTFF profiling available under this axon client)."""
    # enable_asserts / runtime bounds checks emit halt machinery that the axon
    # execution path cannot survive (device goes NRT_EXEC_UNIT_UNRECOVERABLE),
    # so both are disabled; values_load uses skip_runtime_bounds_check.
    nc = bacc.Bacc(
        "TRN2",
        target_bir_lowering=False,
        debug=False,
        num_devices=NCORES,
        enable_asserts=False,
    )
    qt_d = nc.dram_tensor("qt", [BPC, 128, NDC, LQ], BF16, kind="ExternalInput").ap()
    kt_d = nc.dram_tensor("kt", [BPC, 128, NDC, LK], BF16, kind="ExternalInput").ap()
    vp_d = nc.dram_tensor("vp", [BPC, 128, NKB, DV + 1], BF16, kind="ExternalInput").ap()
    mb_d = nc.dram_tensor("mb", [BPC, 128, NKB], F32, kind="ExternalInput").ap()
    if sparse:
        nkb_d = nc.dram_tensor("nkb", [1, BPC], mybir.dt.int32, kind="ExternalInput").ap()
    if timing_loop:
        ri_d = nc.dram_tensor("riter", [1, 1], mybir.dt.int32, kind="ExternalInput").ap()
    out_d = nc.dram_tensor("out", [BPC, LQ // 128, 128, DV], F32, kind="ExternalOutput").ap()

    with tile.TileContext(nc) as tc:
        with (
            tc.tile_pool(name="inp", bufs=2) as inp,
            tc.tile_pool(name="work", bufs=3) as work,
            tc.tile_pool(name="outp", bufs=4) as outp,
            tc.tile_pool(name="psum", bufs=1, space="PSUM") as psum,
            contextlib.ExitStack() as body_cm,
        ):
            if sparse:
                nkb_sb = work.tile([1, BPC], mybir.dt.int32, tag="nkb", bufs=1)
                nc.sync.dma_start(nkb_sb, nkb_d)
            if timing_loop:
                ri_sb = work.tile([1, 1], mybir.dt.int32, tag="ri", bufs=1)
                nc.sync.dma_start(ri_sb, ri_d)
                riter = nc.values_load(
                    ri_sb, min_val=1, max_val=1 << 20, skip_runtime_bounds_check=True
                )
                body_cm.enter_context(tc.For_i(0, riter))
            if sparse:
                # Only PE and ACT have instructions inside the gated groups, so
                # only they need the value (fewer engines -> cheaper If blocks).
                nkb_sv = [
                    nc.values_load(
                        nkb_sb[:, b : b + 1],
                        engines=[mybir.EngineType.PE, mybir.EngineType.Activation],
                        min_val=1,
                        max_val=NKB,
                        skip_runtime_bounds_check=True,
                    )
                    for b in range(BPC)
                ]

            # Preload the exp table set (~2.7us) while the first inputs stream in.
            warm_in = work.tile([128, 1], F32, tag="warm", bufs=1)
            warm_out = work.tile([128, 1], F32, tag="warm2", bufs=1)
            nc.vector.memset(warm_in, 0.0)
            nc.scalar.activation(warm_out, warm_in, EXP, bias=warm_in, scale=1.0)

            loaded = {}

            def load_batch(b):
                qt_sb = inp.tile([128, NDC, LQ], BF16, tag="qt", name="qt_sb")
                kt_sb = inp.tile([128, NDC, LK], BF16, tag="kt", name="kt_sb")
                vp_sb = inp.tile([128, NKB, DV + 1], BF16, tag="vp", name="vp_sb")
                mb_sb = inp.tile([128, NKB], F32, tag="mb", name="mb_sb")
                nc.sync.dma_start(kt_sb, kt_d[b])
                nc.sync.dma_start(qt_sb, qt_d[b])
                nc.sync.dma_start(vp_sb, vp_d[b])
                nc.sync.dma_start(mb_sb, mb_d[b])
                loaded[b] = (qt_sb, kt_sb, vp_sb, mb_sb)

            if prefetch:
                for b in range(BPC):
                    load_batch(b)

            for b in range(BPC):
                if not prefetch:
                    load_batch(b)
                qt_sb, kt_sb, vp_sb, mb_sb = loaded[b]

                for iq in range(NQT):
                    po = [
                        psum.tile(
                            [128, DV + 1], F32, tag=f"po{qs}", bufs=1, name=f"po{qs}"
                        )
                        for qs in range(NQS)
                    ]

                    def kb_body(kb, last_kb):
                        ps = psum.tile([128, QT], F32, tag="ps", bufs=ps_bufs, name="ps")
                        for c in range(NDC):
                            nc.tensor.matmul(
                                ps,
                                kt_sb[:, c, kb * 128 : (kb + 1) * 128],
                                qt_sb[:, c, iq * QT : (iq + 1) * QT],
                                start=(c == 0),
                                stop=(c == NDC - 1),
                            )
                        pt = work.tile([128, QT], BF16, tag="pt", name="pt")
                        nc.scalar.activation(
                            pt, ps, EXP, bias=mb_sb[:, kb : kb + 1], scale=0.0625
                        )
                        for qs in range(NQS):
                            nc.tensor.matmul(
                                po[qs],
                                pt[:, qs * 128 : (qs + 1) * 128],
                                vp_sb[:, kb, :],
                                start=(kb == 0),
                                stop=(kb == last_kb),
                                skip_group_check=sparse,
                            )

                    if not sparse:
                        for kb in range(NKB):
                            kb_body(kb, NKB - 1)
                    else:
                        for g in range(NKB // GS):
                            gate = (
                                contextlib.nullcontext()
                                if g == 0
                                else tc.If(nkb_sv[b] > g * GS)
                            )
                            with gate:
                                for kb in range(g * GS, (g + 1) * GS):
                                    kb_body(kb, (g + 1) * GS - 1)

                    for qs in range(NQS):
                        rec = outp.tile([128, 1], F32, tag="rec")
                        nc.vector.reciprocal(rec, po[qs][:, DV : DV + 1])
                        ob = outp.tile([128, DV], F32, tag="ob")
                        nc.vector.tensor_scalar_mul(ob, po[qs][:, 0:DV], rec)
                        nc.sync.dma_start(out_d[b, iq * NQS + qs], ob)

    nc.compile()
    return nc


def get_program(timing_loop: bool = False, sparse: bool = False, **opts):
    key = (bool(timing_loop), bool(sparse), tuple(sorted(opts.items())))
    if key not in _progs:
        _progs[key] = _build_program(timing_loop=key[0], sparse=key[1], **opts)
    return _progs[key]


def _pack_core_inputs(query, key, value, valid_len, batches):
    bf16 = ml_dtypes.bfloat16
    qt = np.empty((BPC, 128, NDC, LQ), dtype=bf16)
    kt = np.empty((BPC, 128, NDC, LK), dtype=bf16)
    vp = np.empty((BPC, 128, NKB, DV + 1), dtype=bf16)
    mb = np.empty((BPC, 128, NKB), dtype=np.float32)
    nkb = np.zeros((1, BPC), dtype=np.int32)
    karange = np.arange(LK)
    for i, b in enumerate(batches):
        nkb[0, i] = -(-int(valid_len[b]) // 128)
        qt[i] = query[b].T.reshape(NDC, 128, LQ).transpose(1, 0, 2).astype(bf16)
        kt[i] = key[b].T.reshape(NDC, 128, LK).transpose(1, 0, 2).astype(bf16)
        vv = np.concatenate(
            [value[b], np.ones((LK, 1), np.float32)], axis=1
        )  # [LK, DV+1]
        vp[i] = vv.reshape(NKB, 128, DV + 1).transpose(1, 0, 2).astype(bf16)
        bias = np.where(karange < int(valid_len[b]), 0.0, -1e30).astype(np.float32)
        mb[i] = bias.reshape(NKB, 128).T
    return {"qt": qt, "kt": kt, "vp": vp, "mb": mb, "nkb": nkb}


def make_pairs(valid_len):
    """Pair longest-valid with shortest-valid batches per core (load balance)."""
    order = np.argsort(-np.asarray(valid_len).astype(np.int64), kind="stable")
    return [(int(order[i]), int(order[B - 1 - i])) for i in range(NCORES)]


def kernel(query, key, value, valid_len, _res_out=None):
    query = np.asarray(query, dtype=np.float32)
    key = np.asarray(key, dtype=np.float32)
    value = np.asarray(value, dtype=np.float32)
    valid_len = np.asarray(valid_len)

    pairs = make_pairs(valid_len)
    in_maps = [
        _pack_core_inputs(query, key, value, valid_len, pairs[c]) for c in range(NCORES)
    ]

    nc = get_program()
    res = run_bass_kernel_spmd(nc, in_maps, core_ids=list(range(NCORES)))
    if _res_out is not None:
        _res_out.append(res)

    out = np.empty((B, LQ, DV), dtype=np.float32)
    for c in range(NCORES):
        r = np.asarray(res.results[c]["out"], dtype=np.float32)
        for i, b in enumerate(pairs[c]):
            out[b] = r[i].reshape(LQ, DV)
    return out


# revision 41
# speedup vs baseline: 3.4896x; 3.4896x over previous
"""Masked dot-product attention (B=16, Lq=Lk=2048, D=Dv=256, fp32) on 8 trn2 cores.

Strategy (data-parallel over batch, 2 batches/core):
  - Host pre-transposes Q,K to [d, seq] layout (bf16) so both matmuls run
    without any on-device transposes:
      S^T[k,q]  = (K^T chunk).T @ (Q^T chunk)   (contract d, 2 chunks of 128)
      P^T[k,q]  = exp(S^T/16 + maskbias[k])     (ACT, per-partition bias folds
                                                 the valid_len mask: -1e30 -> 0)
      O[q,v]    = sum_kb (P^T slice).T @ V'[kb] (accumulate in PSUM)
    where V' = [V | ones] so column 256 of O accumulates sum_k exp = softmax
    denominator; final normalize is a per-partition reciprocal-multiply.
  - Scores are O(1) (inputs ~N(0,1), /sqrt(256)), so exp without max-subtraction
    is numerically safe in fp32; matmul inputs in bf16 (fp32 PSUM accumulate).
"""

import contextlib
import os

import numpy as np
import ml_dtypes

import concourse.bass as bass
import concourse.bacc as bacc
import concourse.tile as tile
from concourse import mybir
from concourse.bass_utils import run_bass_kernel_spmd

B, LQ, LK, D, DV = 16, 2048, 2048, 256, 256
NCORES = 8
BPC = B // NCORES          # batches per core
NKB = LK // 128            # 16 k-blocks of 128
QT = 512                   # q tile (one PSUM bank of fp32)
NQT = LQ // QT             # 4
NQS = QT // 128            # 4 q sub-tiles per q tile
NDC = D // 128             # 2 contraction chunks

BF16 = mybir.dt.bfloat16
F32 = mybir.dt.float32
EXP = mybir.ActivationFunctionType.Exp

_progs = {}

# Sparse mode: k-block groups past the first are wrapped in runtime
# `If(nkb[b] > g*GS)` so fully-masked tails are skipped on-device.
GS = 4  # k-blocks per gated group


def _build_program(
    timing_loop: bool = False,
    sparse: bool = False,
    ps_bufs: int = 3,
    prefetch: bool = False,
    gs: int = GS,
    hoist: bool = False,
    skip_compute: bool = False,
    skip_out: bool = False,
    out_scalar: bool = False,
    out_bf16: bool = False,
    skip_loads: bool = False,
    pt_bufs: int = 3,
    inp_bufs: int = 2,
    loads_gpsimd: bool = False,
):
    """Build the SPMD program. With timing_loop=True an extra int32 input
    `riter` [1,1] repeats the whole body riter times on-device (wall-clock
    slope timing — no NTFF profiling available under this axon client)."""
    # enable_asserts / runtime bounds checks emit halt machinery that the axon
    # execution path cannot survive (device goes NRT_EXEC_UNIT_UNRECOVERABLE),
    # so both are disabled; values_load uses skip_runtime_bounds_check.
    nc = bacc.Bacc(
        "TRN2",
        target_bir_lowering=False,
        debug=False,
        num_devices=NCORES,
        enable_asserts=False,
    )
    qt_d = nc.dram_tensor("qt", [BPC, 128, NDC, LQ], BF16, kind="ExternalInput").ap()
    kt_d = nc.dram_tensor("kt", [BPC, 128, NDC, LK], BF16, kind="ExternalInput").ap()
    vp_d = nc.dram_tensor("vp", [BPC, 128, NKB, DV + 1], BF16, kind="ExternalInput").ap()
    mb_d = nc.dram_tensor("mb", [BPC, 128, NKB], F32, kind="ExternalInput").ap()
    if sparse:
        nkb_d = nc.dram_tensor("nkb", [1, BPC], mybir.dt.int32, kind="ExternalInput").ap()
    if timing_loop:
        ri_d = nc.dram_tensor("riter", [1, 1], mybir.dt.int32, kind="ExternalInput").ap()
    out_dt = BF16 if out_bf16 else F32
    out_d = nc.dram_tensor(
        "out", [BPC, LQ // 128, 128, DV], out_dt, kind="ExternalOutput"
    ).ap()

    with tile.TileContext(nc) as tc:
        with (
            tc.tile_pool(name="inp", bufs=inp_bufs) as inp,
            tc.tile_pool(name="work", bufs=3) as work,
            tc.tile_pool(name="outp", bufs=4) as outp,
            tc.tile_pool(name="psum", bufs=1, space="PSUM") as psum,
            contextlib.ExitStack() as body_cm,
        ):
            if sparse:
                nkb_sb = work.tile([1, BPC], mybir.dt.int32, tag="nkb", bufs=1)
                nc.sync.dma_start(nkb_sb, nkb_d)
            if timing_loop:
                ri_sb = work.tile([1, 1], mybir.dt.int32, tag="ri", bufs=1)
                nc.sync.dma_start(ri_sb, ri_d)
                riter = nc.values_load(
                    ri_sb, min_val=1, max_val=1 << 20, skip_runtime_bounds_check=True
                )
                body_cm.enter_context(tc.For_i(0, riter))
            if sparse:
                # Only engines with instructions inside the gated groups need
                # the value (fewer engines -> cheaper If blocks).
                gate_engines = [mybir.EngineType.PE, mybir.EngineType.Activation]
                if hoist:
                    gate_engines.append(mybir.EngineType.DVE)
                nkb_sv = [
                    nc.values_load(
                        nkb_sb[:, b : b + 1],
                        engines=gate_engines,
                        min_val=1,
                        max_val=NKB,
                        skip_runtime_bounds_check=True,
                    )
                    for b in range(BPC)
                ]
                nkb_dma = None
                if skip_loads:
                    nkb_dma = [
                        nc.values_load(
                            nkb_sb[:, b : b + 1],
                            engines=[mybir.EngineType.SP],
                            min_val=1,
                            max_val=NKB,
                            skip_runtime_bounds_check=True,
                        )
                        for b in range(BPC)
                    ]

            # Preload the exp table set (~2.7us) while the first inputs stream in.
            warm_in = work.tile([128, 1], F32, tag="warm", bufs=1)
            warm_out = work.tile([128, 1], F32, tag="warm2", bufs=1)
            nc.vector.memset(warm_in, 0.0)
            nc.scalar.activation(warm_out, warm_in, EXP, bias=warm_in, scale=1.0)

            loaded = {}

            def load_batch(b):
                # Inputs staged as split tiles so the first matmuls only wait
                # on the slices they read, not whole-tensor DMAs.
                mb_sb = inp.tile([128, NKB], F32, tag="mb", name="mb_sb")
                nc.sync.dma_start(mb_sb, mb_d[b])
                kt_sp = []
                qt_sp = []
                vp_sp = []
                for j in range(4):
                    cond = None
                    if skip_loads and j > 0:
                        cond = nkb_dma[b] > j * 4
                    kt_j = inp.tile([128, NDC, LK // 4], BF16, tag=f"kt{j}", name=f"kt{j}")
                    nc.sync.dma_start(
                        kt_j, kt_d[b][:, :, j * (LK // 4) : (j + 1) * (LK // 4)],
                        cond=cond,
                    )
                    kt_sp.append(kt_j)
                    qt_j = inp.tile([128, NDC, QT], BF16, tag=f"qt{j}", name=f"qt{j}")
                    nc.sync.dma_start(qt_j, qt_d[b][:, :, j * QT : (j + 1) * QT])
                    qt_sp.append(qt_j)
                    vp_j = inp.tile(
                        [128, NKB // 4, DV + 1], BF16, tag=f"vp{j}", name=f"vp{j}"
                    )
                    (nc.gpsimd if loads_gpsimd else nc.sync).dma_start(
                        vp_j, vp_d[b][:, (NKB // 4) * j : (NKB // 4) * (j + 1), :],
                        cond=cond,
                    )
                    vp_sp.append(vp_j)
                loaded[b] = (qt_sp, kt_sp, vp_sp, mb_sb)

            if prefetch:
                for b in range(BPC):
                    load_batch(b)

            for b in range(BPC):
                if not prefetch:
                    load_batch(b)
                qt_sp, kt_sp, vp_sp, mb_sb = loaded[b]

                def kb_body(iq, po, kb, first_kb, last_kb):
                    ps = psum.tile([128, QT], F32, tag="ps", bufs=ps_bufs, name="ps")
                    kt_j = kt_sp[kb // 4]
                    kcol = (kb % 4) * 128
                    for c in range(NDC):
                        nc.tensor.matmul(
                            ps,
                            kt_j[:, c, kcol : kcol + 128],
                            qt_sp[iq][:, c, :],
                            start=(c == 0),
                            stop=(c == NDC - 1),
                        )
                    pt = work.tile([128, QT], BF16, tag="pt", bufs=pt_bufs, name="pt")
                    nc.scalar.activation(
                        pt, ps, EXP, bias=mb_sb[:, kb : kb + 1], scale=0.0625
                    )
                    for qs in range(NQS):
                        nc.tensor.matmul(
                            po[qs],
                            pt[:, qs * 128 : (qs + 1) * 128],
                            vp_sp[kb // 4][:, kb % 4, :],
                            start=(kb == first_kb),
                            stop=(kb == last_kb),
                            skip_group_check=sparse,
                        )

                def finish(src_ap, b, j, utag=False):
                    if skip_out:
                        return
                    rtag, otag = (f"rec{j}", f"ob{j}") if utag else ("rec", "ob")
                    rec = outp.tile([128, 1], F32, tag=rtag, name="rec")
                    nc.vector.reciprocal(rec, src_ap[:, DV : DV + 1])
                    ob = outp.tile([128, DV], out_dt, tag=otag, name="ob")
                    nc.vector.tensor_scalar_mul(ob, src_ap[:, 0:DV], rec)
                    # out DMAs on the ACT HWDGE ring so they never queue ahead
                    # of the next batch's input loads on the sync ring
                    (nc.scalar if out_scalar else nc.sync).dma_start(out_d[b, j], ob)

                if skip_compute:
                    dummy = outp.tile([128, DV], F32, tag="dummy", bufs=1, name="dummy")
                    nc.vector.memset(dummy, 1.0)
                    for j in range(NQT * NQS):
                        nc.sync.dma_start(out_d[b, j], dummy)
                    continue

                if sparse and hoist:
                    acc = [
                        work.tile([128, DV + 1], F32, tag=f"acc{j}", bufs=2, name=f"acc{j}")
                        for j in range(NQT * NQS)
                    ]
                    for g in range(NKB // gs):
                        gate = (
                            contextlib.nullcontext()
                            if g == 0
                            else tc.If(nkb_sv[b] > g * gs)
                        )
                        with gate:
                            for iq in range(NQT):
                                po = [
                                    psum.tile(
                                        [128, DV + 1], F32, tag=f"po{qs}", bufs=1,
                                        name=f"po{qs}",
                                    )
                                    for qs in range(NQS)
                                ]
                                for kb in range(g * gs, (g + 1) * gs):
                                    kb_body(iq, po, kb, g * gs, (g + 1) * gs - 1)
                                for qs in range(NQS):
                                    j = iq * NQS + qs
                                    if g == 0:
                                        nc.vector.tensor_copy(acc[j], po[qs])
                                    else:
                                        nc.vector.tensor_add(acc[j], acc[j], po[qs])
                    for j in range(NQT * NQS):
                        finish(acc[j], b, j, utag=True)
                else:
                    for iq in range(NQT):
                        po = [
                            psum.tile(
                                [128, DV + 1], F32, tag=f"po{qs}", bufs=1, name=f"po{qs}"
                            )
                            for qs in range(NQS)
                        ]
                        if not sparse:
                            for kb in range(NKB):
                                kb_body(iq, po, kb, 0, NKB - 1)
                        else:
                            for g in range(NKB // gs):
                                gate = (
                                    contextlib.nullcontext()
                                    if g == 0
                                    else tc.If(nkb_sv[b] > g * gs)
                                )
                                with gate:
                                    for kb in range(g * gs, (g + 1) * gs):
                                        kb_body(iq, po, kb, 0, (g + 1) * gs - 1)
                        for qs in range(NQS):
                            finish(po[qs], b, iq * NQS + qs)

    nc.compile()
    return nc


# Best-measured configuration (graded path): runtime If-gated k-block groups
# (gs=4), all input DMAs issued before any output DMA enters the sync ring.
BEST = dict(sparse=True, prefetch=True)


def get_program(timing_loop: bool = False, sparse: bool = False, **opts):
    key = (bool(timing_loop), bool(sparse), tuple(sorted(opts.items())))
    if key not in _progs:
        _progs[key] = _build_program(timing_loop=key[0], sparse=key[1], **opts)
    return _progs[key]


def _pack_core_inputs(query, key, value, valid_len, batches):
    bf16 = ml_dtypes.bfloat16
    qt = np.empty((BPC, 128, NDC, LQ), dtype=bf16)
    kt = np.empty((BPC, 128, NDC, LK), dtype=bf16)
    vp = np.empty((BPC, 128, NKB, DV + 1), dtype=bf16)
    mb = np.empty((BPC, 128, NKB), dtype=np.float32)
    nkb = np.zeros((1, BPC), dtype=np.int32)
    karange = np.arange(LK)
    for i, b in enumerate(batches):
        nkb[0, i] = -(-int(valid_len[b]) // 128)
        qt[i] = query[b].T.reshape(NDC, 128, LQ).transpose(1, 0, 2).astype(bf16)
        kt[i] = key[b].T.reshape(NDC, 128, LK).transpose(1, 0, 2).astype(bf16)
        vv = np.concatenate(
            [value[b], np.ones((LK, 1), np.float32)], axis=1
        )  # [LK, DV+1]
        vp[i] = vv.reshape(NKB, 128, DV + 1).transpose(1, 0, 2).astype(bf16)
        bias = np.where(karange < int(valid_len[b]), 0.0, -1e30).astype(np.float32)
        mb[i] = bias.reshape(NKB, 128).T
    return {"qt": qt, "kt": kt, "vp": vp, "mb": mb, "nkb": nkb}


def make_pairs(valid_len):
    """Pair longest-valid with shortest-valid batches per core (load balance)."""
    order = np.argsort(-np.asarray(valid_len).astype(np.int64), kind="stable")
    return [(int(order[i]), int(order[B - 1 - i])) for i in range(NCORES)]


def kernel(query, key, value, valid_len, _res_out=None):
    query = np.asarray(query, dtype=np.float32)
    key = np.asarray(key, dtype=np.float32)
    value = np.asarray(value, dtype=np.float32)
    valid_len = np.asarray(valid_len)

    pairs = make_pairs(valid_len)
    in_maps = [
        _pack_core_inputs(query, key, value, valid_len, pairs[c]) for c in range(NCORES)
    ]

    nc = get_program(**BEST)
    res = run_bass_kernel_spmd(nc, in_maps, core_ids=list(range(NCORES)))
    if _res_out is not None:
        _res_out.append(res)

    out = np.empty((B, LQ, DV), dtype=np.float32)
    for c in range(NCORES):
        r = np.asarray(res.results[c]["out"], dtype=np.float32)
        for i, b in enumerate(pairs[c]):
            out[b] = r[i].reshape(LQ, DV)
    return out
